# revision 2
# baseline (speedup 1.0000x reference)
"""Grouped Conv1d (B=4, T=512, G=129, F=96 -> O=96, K=3, pad=1) on 8 trn2 cores.

Sharding: 129 groups = 16 full groups per core + group 128 split across all
8 cores by (batch b = core//2, T-half = core%2).  SPMD: every core runs the
identical program on its own slice.

Per (group, batch): out[o, t] = sum_k w_k[f, o].T @ x[f, t+k-1]  (3 matmuls
accumulated in fp32 PSUM).  Bias is folded into the matmul as contraction
row 96 (x row 96 = 1.0, w row 96 = bias on the k==0 slot), so the
PSUM -> SBUF drain is a pure fp32->fp16 cast copy on ScalarE/VectorE.
x and w are fp16: full-rate PE, half the DMA bytes, max rel err ~5e-4.

DMA strategy: per-packet cost is ~9-16 ns regardless of size and packet
size = innermost contiguous run, so all x transfers are whole-group
(4112 B rows) or whole-batch (8224 B rows) to ride at ~300 GB/s.  Loads
and stores alternate between the two HWDGE rings (SP + ACT); weights for
groups 4+ and the tail ride gpsimd's SWDGE so they never block the x
stream.  Matmuls run b-outer/k-inner so each PSUM tile retires ASAP; the
last group stores per-b so the tail drain is one small transfer.
"""

from contextlib import ExitStack

import numpy as np

import concourse.bass as bass
import concourse.mybir as mybir
import concourse.tile as tile
from concourse import bacc
from concourse.bass_utils import run_bass_kernel_spmd

B, T, G, F, O, K = 4, 512, 129, 96, 96, 3
NCORES = 8
GPC = 16          # full groups per core (8*16 = 128; group 128 is split 8 ways)
NG = GPC + 1      # per-core group slots incl. the shared tail group
TP = T + 2        # T padded by K//2 on both sides
TE = T // 2       # tail-group T chunk per core
TEP = TE + 2
GB = 2            # groups per x batch
NB = GPC // GB
FB = F + 1        # contraction rows: 96 features + 1 bias row


def build_program():
    nc = bacc.Bacc("TRN2", target_bir_lowering=False, debug=False,
                   num_devices=NCORES)

    f32 = mybir.dt.float32
    f16 = mybir.dt.float16

    xm = nc.dram_tensor("xm", [NB, FB, GB, B, TP], f16, kind="ExternalInput")
    xe = nc.dram_tensor("xe", [FB, TEP], f16, kind="ExternalInput")
    wt = nc.dram_tensor("wt", [FB, NG * K * O], f16, kind="ExternalInput")
    om = nc.dram_tensor("om", [NB, O, GB, B, T], f16, kind="ExternalOutput")
    oe = nc.dram_tensor("oe", [O, TE], f16, kind="ExternalOutput")

    with ExitStack() as ctx:
        tc = ctx.enter_context(tile.TileContext(nc))
        wpool = ctx.enter_context(tc.tile_pool(name="w", bufs=1))
        xpool = ctx.enter_context(tc.tile_pool(name="x", bufs=5))
        opool = ctx.enter_context(tc.tile_pool(name="o", bufs=4))
        pspool = ctx.enter_context(tc.tile_pool(name="ps", bufs=8, space="PSUM"))

        w_sb = wpool.tile([FB, NG * K * O], f16)
        xe_sb = wpool.tile([FB, TEP], f16)
        oe_sb = wpool.tile([O, TE], f16)

        kwc = K * O                 # w elems per group per partition row
        XG = B * TP                 # x elems per group per partition row

        x_tiles = {}

        def xtile(ib):
            t_ = xpool.tile([FB, GB * XG], f16, tag="x", name=f"x{ib}")
            x_tiles[ib] = t_
            return t_

        xm_f = [xm[i].rearrange("f g b t -> f (g b t)") for i in range(NB)]
        x0, x1, x2 = xtile(0), xtile(1), xtile(2)

        # prologue: the critical pair (w groups 0-1, x group 0) first on the
        # two HW rings, then progressively larger pieces; bulk weights on
        # gpsimd's SWDGE so they never delay the x stream
        nc.scalar.dma_start(x0[:, :XG], xm_f[0][:, :XG])                # g0
        nc.sync.dma_start(w_sb[:, :2 * kwc], wt[:, :2 * kwc])           # w g0-1
        nc.scalar.dma_start(x1[:], xm_f[1][:])                          # batch 1
        nc.sync.dma_start(w_sb[:, 2 * kwc:4 * kwc],
                          wt[:, 2 * kwc:4 * kwc])                       # w g2-3
        nc.sync.dma_start(x0[:, XG:], xm_f[0][:, XG:])                  # g1
        nc.sync.dma_start(x2[:], xm_f[2][:])                            # batch 2
        nc.sync.dma_start(w_sb[:, 4 * kwc:8 * kwc],
                          wt[:, 4 * kwc:8 * kwc])                       # w g4-7
        nc.gpsimd.dma_start(xe_sb[:], xe[:])
        nc.gpsimd.dma_start(w_sb[:, 8 * kwc:], wt[:, 8 * kwc:])         # w g8-16

        for ib in range(NB):
            if 3 <= ib + 2 < NB:
                e = nc.scalar if (ib + 2) % 2 == 1 else nc.sync
                e.dma_start(xtile(ib + 2)[:], xm_f[ib + 2][:])
            x_sb = x_tiles.pop(ib)
            om_f = om[ib].rearrange("o g b t -> o (g b t)")
            for j in range(GB):
                g = ib * GB + j
                o_sb = opool.tile([O, B * T], f16, tag="o")
                for b in range(B):
                    ps = pspool.tile([O, T], f32, tag="ps")
                    for k in range(K):
                        nc.tensor.matmul(
                            ps[:],
                            w_sb[:, (g * K + k) * O:(g * K + k + 1) * O],
                            x_sb[:, (j * B + b) * TP + k:
                                 (j * B + b) * TP + k + T],
                            start=(k == 0),
                            stop=(k == K - 1),
                        )
                    dst = o_sb[:, b * T:(b + 1) * T]
                    if g == GPC - 1 and b == B - 1:
                        # final tile: split the cast across both engines so
                        # the drain after the last matmul is minimal
                        nc.scalar.copy(dst[:, :T // 2], ps[:, :T // 2])
                        nc.vector.tensor_scalar_add(dst[:, T // 2:],
                                                    ps[:, T // 2:], 0.0)
                    elif b % 2 == 0:
                        nc.scalar.copy(dst, ps[:])
                    else:
                        nc.vector.tensor_scalar_add(dst, ps[:], 0.0)
                if g < GPC - 1:
                    e = nc.sync if g % 2 == 0 else nc.scalar
                    e.dma_start(om_f[:, j * B * T:(j + 1) * B * T], o_sb[:])
                else:
                    # last group: store b0-b2 as soon as their casts land,
                    # b3 alone so the post-matmul drain is one small DMA
                    c0 = j * B * T
                    nc.sync.dma_start(om_f[:, c0:c0 + 2 * T], o_sb[:, :2 * T])
                    nc.scalar.dma_start(om_f[:, c0 + 2 * T:c0 + 3 * T],
                                        o_sb[:, 2 * T:3 * T])
                    nc.sync.dma_start(om_f[:, c0 + 3 * T:c0 + 4 * T],
                                      o_sb[:, 3 * T:])
            if ib == 3:
                # tail group (g=128): tiny, mid-stream, output via SWDGE
                ps = pspool.tile([O, TE], f32, tag="ps")
                for k in range(K):
                    nc.tensor.matmul(
                        ps[:],
                        w_sb[:, (GPC * K + k) * O:(GPC * K + k + 1) * O],
                        xe_sb[:, k:k + TE],
                        start=(k == 0),
                        stop=(k == K - 1),
                    )
                nc.vector.tensor_scalar_add(oe_sb[:], ps[:], 0.0)
                nc.gpsimd.dma_start(oe[:], oe_sb[:])

    nc.finalize()
    return nc


def shard_inputs(x, weight, bias):
    x = np.ascontiguousarray(x, dtype=np.float32)
    weight = np.ascontiguousarray(weight, dtype=np.float32)
    bias = np.ascontiguousarray(bias, dtype=np.float32)

    xp = np.pad(x, ((0, 0), (1, 1), (0, 0), (0, 0)))          # [B, TP, G, F]
    xt = xp.transpose(2, 3, 0, 1).astype(np.float16)          # [G, F, B, TP]
    # weight [G, O, F, K] -> [F, G, K, O]
    wtr = weight.transpose(2, 0, 3, 1).astype(np.float16)

    in_maps = []
    for c in range(NCORES):
        gs = list(range(c * GPC, (c + 1) * GPC)) + [G - 1]
        b_c, t0 = c // 2, (c % 2) * TE
        # x: [GPC, F, B, TP] -> [NB, F, GB, B, TP], then append the ones row
        xm_c = xt[c * GPC:(c + 1) * GPC].reshape(NB, GB, F, B, TP)
        xm_c = xm_c.transpose(0, 2, 1, 3, 4)
        xm_c = np.concatenate(
            [xm_c, np.ones((NB, 1, GB, B, TP), dtype=np.float16)], axis=1)
        # w: [F, NG, K, O] + bias row (k==0 slot only)
        wt_c = np.zeros((FB, NG, K, O), dtype=np.float16)
        wt_c[:F] = wtr[:, gs]
        wt_c[F, :, 0, :] = bias[gs].astype(np.float16)
        # tail x chunk + ones row
        xe_c = np.empty((FB, TEP), dtype=np.float16)
        xe_c[:F] = xt[G - 1, :, b_c, t0:t0 + TEP]
        xe_c[F] = 1.0
        in_maps.append({
            "xm": np.ascontiguousarray(xm_c),
            "xe": xe_c,
            "wt": np.ascontiguousarray(wt_c.reshape(FB, NG * K * O)),
            })
    return in_maps


def unshard_outputs(results):
    out = np.empty((B, T, G, O), dtype=np.float32)
    for c in range(NCORES):
        om = results[c]["om"].astype(np.float32)        # [NB, O, GB, B, T]
        om = om.transpose(0, 2, 1, 3, 4).reshape(GPC, O, B, T)
        out[:, :, c * GPC:(c + 1) * GPC, :] = om.transpose(2, 3, 0, 1)
        b_c, t0 = c // 2, (c % 2) * TE
        out[b_c, t0:t0 + TE, G - 1, :] = results[c]["oe"].astype(np.float32).T
    return out


def run(x, weight, bias, **run_kwargs):
    nc = build_program()
    in_maps = shard_inputs(x, weight, bias)
    res = run_bass_kernel_spmd(nc, in_maps, list(range(NCORES)), **run_kwargs)
    return unshard_outputs(res.results), res


def kernel(x, weight, bias):
    out, _ = run(x, weight, bias)
    return out


# revision 3
# speedup vs baseline: 3.8922x; 3.8922x over previous
"""Grouped Conv1d (B=4, T=512, G=129, F=96 -> O=96, K=3, pad=1) on 8 trn2 cores.

Sharding: 129 groups = 16 full groups per core + group 128 split across all
8 cores by (batch b = core//2, T-half = core%2).  SPMD: every core runs the
identical program on its own slice.

Per (group, batch): out[o, t] = sum_k w_k[f, o].T @ x[f, t+k-1]  (3 matmuls
accumulated in fp32 PSUM).  Bias is folded into the matmul as contraction
row 96 (x row 96 = 1.0, w row 96 = bias on the k==0 slot), so the
PSUM -> SBUF drain is a pure fp32->fp16 cast copy on ScalarE/VectorE.
x and w are fp16: full-rate PE, half the DMA bytes, max rel err ~5e-4.

DMA strategy: per-packet cost is roughly fixed (~10-15 ns) and packet size
equals the innermost contiguous run, so bulk x transfers use 2056-8224 B
rows to ride at ~300 GB/s.  DMAs with 97 partitions hit a pathological
slow path in the DMA engine, so every transfer is <=96 partitions: bulk
rows 0-95, and the ones/bias row 96 via separate 1-partition transfers.
Loads and stores alternate between the two HWDGE rings (SP + ACT); bulk
weights for groups 8+ ride gpsimd's SWDGE so they never block the x
stream.  Matmuls run b-outer/k-inner so each PSUM tile retires ASAP; the
last group stores per-b so the tail drain is one small transfer.
"""

from contextlib import ExitStack

import numpy as np

import concourse.bass as bass
import concourse.mybir as mybir
import concourse.tile as tile
from concourse import bacc
from concourse.bass_utils import run_bass_kernel_spmd

B, T, G, F, O, K = 4, 512, 129, 96, 96, 3
NCORES = 8
GPC = 16          # full groups per core (8*16 = 128; group 128 is split 8 ways)
NG = GPC + 1      # per-core group slots incl. the shared tail group
TP = T + 2        # T padded by K//2 on both sides
TE = T // 2       # tail-group T chunk per core
TEP = TE + 2
GB = 2            # groups per x batch
NB = GPC // GB
FB = F + 1        # contraction rows: 96 features + 1 bias row
NXB = 5           # x double-buffer depth


def build_program():
    nc = bacc.Bacc("TRN2", target_bir_lowering=False, debug=False,
                   num_devices=NCORES)

    f32 = mybir.dt.float32
    f16 = mybir.dt.float16

    xm = nc.dram_tensor("xm", [NB, FB, GB, B, TP], f16, kind="ExternalInput")
    xe = nc.dram_tensor("xe", [FB, TEP], f16, kind="ExternalInput")
    wt = nc.dram_tensor("wt", [FB, NG * K * O], f16, kind="ExternalInput")
    om = nc.dram_tensor("om", [NB, O, GB, B, T], f16, kind="ExternalOutput")
    oe = nc.dram_tensor("oe", [O, TE], f16, kind="ExternalOutput")

    kwc = K * O                 # w elems per group per partition row
    XG = B * TP                 # x elems per group per partition row

    with ExitStack() as ctx:
        tc = ctx.enter_context(tile.TileContext(nc))
        wpool = ctx.enter_context(tc.tile_pool(name="w", bufs=1))
        opool = ctx.enter_context(tc.tile_pool(name="o", bufs=4))
        pspool = ctx.enter_context(tc.tile_pool(name="ps", bufs=8, space="PSUM"))

        w_sb = wpool.tile([FB, NG * K * O], f16)
        xe_sb = wpool.tile([FB, TEP], f16)
        oe_sb = wpool.tile([O, TE], f16)
        # static x buffers, rotated manually (batch ib -> xbufs[ib % NXB]);
        # row 96 (the ones row) is written once at startup
        xbufs = [wpool.tile([FB, GB * XG], f16, name=f"xb{i}")
                 for i in range(NXB)]

        xm_f = [xm[i].rearrange("f g b t -> f (g b t)") for i in range(NB)]

        # prologue.  gpsimd (SWDGE): ones rows, tail x, bulk weights.
        for i in range(NXB):
            nc.gpsimd.dma_start(xbufs[i][F:FB, :], xm_f[0][F:FB, :])
        nc.gpsimd.dma_start(xe_sb[:F, :], xe[:F, :])
        nc.gpsimd.dma_start(xe_sb[F:FB, :], xe[F:FB, :])
        nc.gpsimd.dma_start(w_sb[:F, 8 * kwc:], wt[:F, 8 * kwc:])      # w g8-16
        # HW rings: critical pair (w g0-1 + x g0 halves) first.
        nc.scalar.dma_start(xbufs[0][:F, :XG // 2], xm_f[0][:F, :XG // 2])
        nc.sync.dma_start(w_sb[:F, :2 * kwc], wt[:F, :2 * kwc])        # w g0-1
        nc.sync.dma_start(w_sb[F:FB, :], wt[F:FB, :])                  # bias row
        nc.scalar.dma_start(xbufs[0][:F, XG // 2:XG],
                            xm_f[0][:F, XG // 2:XG])
        nc.sync.dma_start(w_sb[:F, 2 * kwc:4 * kwc],
                          wt[:F, 2 * kwc:4 * kwc])                     # w g2-3
        nc.scalar.dma_start(xbufs[1][:F, :], xm_f[1][:F, :])           # batch 1
        nc.sync.dma_start(xbufs[0][:F, XG:], xm_f[0][:F, XG:])         # g1
        nc.sync.dma_start(xbufs[2][:F, :], xm_f[2][:F, :])             # batch 2
        nc.sync.dma_start(w_sb[:F, 4 * kwc:8 * kwc],
                          wt[:F, 4 * kwc:8 * kwc])                     # w g4-7

        for ib in range(NB):
            if 3 <= ib + 2 < NB:
                e = nc.scalar if (ib + 2) % 2 == 1 else nc.sync
                e.dma_start(xbufs[(ib + 2) % NXB][:F, :], xm_f[ib + 2][:F, :])
            x_sb = xbufs[ib % NXB]
            om_f = om[ib].rearrange("o g b t -> o (g b t)")
            for j in range(GB):
                g = ib * GB + j
                o_sb = opool.tile([O, B * T], f16, tag="o")
                for b in range(B):
                    ps = pspool.tile([O, T], f32, tag="ps")
                    for k in range(K):
                        nc.tensor.matmul(
                            ps[:],
                            w_sb[:, (g * K + k) * O:(g * K + k + 1) * O],
                            x_sb[:, (j * B + b) * TP + k:
                                 (j * B + b) * TP + k + T],
                            start=(k == 0),
                            stop=(k == K - 1),
                        )
                    dst = o_sb[:, b * T:(b + 1) * T]
                    if g == GPC - 1 and b == B - 1:
                        # final tile: split the cast across both engines so
                        # the drain after the last matmul is minimal
                        nc.scalar.copy(dst[:, :T // 2], ps[:, :T // 2])
                        nc.vector.tensor_scalar_add(dst[:, T // 2:],
                                                    ps[:, T // 2:], 0.0)
                    elif b % 2 == 0:
                        nc.scalar.copy(dst, ps[:])
                    else:
                        nc.vector.tensor_scalar_add(dst, ps[:], 0.0)
                if g < GPC - 1:
                    e = nc.sync if g % 2 == 0 else nc.scalar
                    e.dma_start(om_f[:, j * B * T:(j + 1) * B * T], o_sb[:])
                else:
                    # last group: store b0-b2 as soon as their casts land,
                    # b3 alone so the post-matmul drain is one small DMA
                    c0 = j * B * T
                    nc.sync.dma_start(om_f[:, c0:c0 + 2 * T], o_sb[:, :2 * T])
                    nc.scalar.dma_start(om_f[:, c0 + 2 * T:c0 + 3 * T],
                                        o_sb[:, 2 * T:3 * T])
                    nc.sync.dma_start(om_f[:, c0 + 3 * T:c0 + 4 * T],
                                      o_sb[:, 3 * T:])
            if ib == 3:
                # tail group (g=128): tiny, mid-stream, output via SWDGE
                ps = pspool.tile([O, TE], f32, tag="ps")
                for k in range(K):
                    nc.tensor.matmul(
                        ps[:],
                        w_sb[:, (GPC * K + k) * O:(GPC * K + k + 1) * O],
                        xe_sb[:, k:k + TE],
                        start=(k == 0),
                        stop=(k == K - 1),
                    )
                nc.vector.tensor_scalar_add(oe_sb[:], ps[:], 0.0)
                nc.gpsimd.dma_start(oe[:], oe_sb[:])

    nc.finalize()
    return nc


def shard_inputs(x, weight, bias):
    x = np.ascontiguousarray(x, dtype=np.float32)
    weight = np.ascontiguousarray(weight, dtype=np.float32)
    bias = np.ascontiguousarray(bias, dtype=np.float32)

    xp = np.pad(x, ((0, 0), (1, 1), (0, 0), (0, 0)))          # [B, TP, G, F]
    xt = xp.transpose(2, 3, 0, 1).astype(np.float16)          # [G, F, B, TP]
    # weight [G, O, F, K] -> [F, G, K, O]
    wtr = weight.transpose(2, 0, 3, 1).astype(np.float16)

    in_maps = []
    for c in range(NCORES):
        gs = list(range(c * GPC, (c + 1) * GPC)) + [G - 1]
        b_c, t0 = c // 2, (c % 2) * TE
        # x: [GPC, F, B, TP] -> [NB, F, GB, B, TP], then append the ones row
        xm_c = xt[c * GPC:(c + 1) * GPC].reshape(NB, GB, F, B, TP)
        xm_c = xm_c.transpose(0, 2, 1, 3, 4)
        xm_c = np.concatenate(
            [xm_c, np.ones((NB, 1, GB, B, TP), dtype=np.float16)], axis=1)
        # w: [F, NG, K, O] + bias row (k==0 slot only)
        wt_c = np.zeros((FB, NG, K, O), dtype=np.float16)
        wt_c[:F] = wtr[:, gs]
        wt_c[F, :, 0, :] = bias[gs].astype(np.float16)
        # tail x chunk + ones row
        xe_c = np.empty((FB, TEP), dtype=np.float16)
        xe_c[:F] = xt[G - 1, :, b_c, t0:t0 + TEP]
        xe_c[F] = 1.0
        in_maps.append({
            "xm": np.ascontiguousarray(xm_c),
            "xe": xe_c,
            "wt": np.ascontiguousarray(wt_c.reshape(FB, NG * K * O)),
            })
    return in_maps


def unshard_outputs(results):
    out = np.empty((B, T, G, O), dtype=np.float32)
    for c in range(NCORES):
        om = results[c]["om"].astype(np.float32)        # [NB, O, GB, B, T]
        om = om.transpose(0, 2, 1, 3, 4).reshape(GPC, O, B, T)
        out[:, :, c * GPC:(c + 1) * GPC, :] = om.transpose(2, 3, 0, 1)
        b_c, t0 = c // 2, (c % 2) * TE
        out[b_c, t0:t0 + TE, G - 1, :] = results[c]["oe"].astype(np.float32).T
    return out


def run(x, weight, bias, **run_kwargs):
    nc = build_program()
    in_maps = shard_inputs(x, weight, bias)
    res = run_bass_kernel_spmd(nc, in_maps, list(range(NCORES)), **run_kwargs)
    return unshard_outputs(res.results), res


def kernel(x, weight, bias):
    out, _ = run(x, weight, bias)
    return out


# revision 4
# speedup vs baseline: 3.9993x; 1.0275x over previous
"""Grouped Conv1d (B=4, T=512, G=129, F=96 -> O=96, K=3, pad=1) on 8 trn2 cores.

Sharding: 129 groups = 16 full groups per core + group 128 split across all
8 cores by (batch b = core//2, T-half = core%2).  SPMD: every core runs the
identical program on its own slice.

Per (group, batch): out[o, t] = sum_k w_k[f, o].T @ x[f, t+k-1]  (3 matmuls
accumulated in fp32 PSUM), ordered b-outer/k-inner so each PSUM tile
retires as soon as its 3 matmuls finish.  x and w are cast to fp16 on the
host: full-rate PE, half the DMA bytes, max rel err ~5e-4.  Bias is added
fp32 on ScalarE/VectorE (alternating) while casting PSUM -> SBUF fp16.

DMA strategy: the HWDGE rings split every transfer into 16 descriptors and
FAIR-SHARE bandwidth across all outstanding descriptors (not FIFO), and
per-packet cost is roughly fixed with packet size = innermost contiguous
run.  So: the critical first transfers (w g0-1, x g0's b0+b1) get the two
rings to themselves; the second half of g0 is serialized behind the first
via a deliberate 1-element overlap (WAW dep); bulk weights ride gpsimd's
SWDGE; batch loads are issued from inside the compute loop so their ring
occupancy trails the critical stream.  Stores are per-group (4096 B rows),
and the kernel ends with the tiny tail group so the final drain is one
[96,256] cast + 49 KB store.
"""

from contextlib import ExitStack

import numpy as np

import concourse.bass as bass
import concourse.mybir as mybir
import concourse.tile as tile
from concourse import bacc
from concourse.bass_utils import run_bass_kernel_spmd

B, T, G, F, O, K = 4, 512, 129, 96, 96, 3
NCORES = 8
GPC = 16          # full groups per core (8*16 = 128; group 128 is split 8 ways)
NG = GPC + 1      # per-core group slots incl. the shared tail group
TP = T + 2        # T padded by K//2 on both sides
TE = T // 2       # tail-group T chunk per core
TEP = TE + 2
GB = 2            # groups per x batch
NB = GPC // GB
NXB = 5           # x buffer rotation depth


def build_program():
    nc = bacc.Bacc("TRN2", target_bir_lowering=False, debug=False,
                   num_devices=NCORES)

    f32 = mybir.dt.float32
    f16 = mybir.dt.float16

    xm = nc.dram_tensor("xm", [NB, F, GB, B, TP], f16, kind="ExternalInput")
    xe = nc.dram_tensor("xe", [F, TEP], f16, kind="ExternalInput")
    wt = nc.dram_tensor("wt", [F, NG * K * O], f16, kind="ExternalInput")
    bt = nc.dram_tensor("bt", [O, NG], f32, kind="ExternalInput")
    om = nc.dram_tensor("om", [NB, O, GB, B, T], f16, kind="ExternalOutput")
    oe = nc.dram_tensor("oe", [O, TE], f16, kind="ExternalOutput")

    kwc = K * O                 # w elems per group per partition row
    XG = B * TP                 # x elems per group per partition row
    XU = TP * 2                 # two batch-units (b0+b1) of one group

    with ExitStack() as ctx:
        tc = ctx.enter_context(tile.TileContext(nc))
        wpool = ctx.enter_context(tc.tile_pool(name="w", bufs=1))
        opool = ctx.enter_context(tc.tile_pool(name="o", bufs=4))
        pspool = ctx.enter_context(tc.tile_pool(name="ps", bufs=8, space="PSUM"))

        w_sb = wpool.tile([F, NG * K * O], f16)
        b_sb = wpool.tile([O, NG], f32)
        xe_sb = wpool.tile([F, TEP], f16)
        oe_sb = wpool.tile([O, TE], f16)
        # static x buffers, rotated manually (batch ib -> xbufs[ib % NXB])
        xbufs = [wpool.tile([F, GB * XG], f16, name=f"xb{i}")
                 for i in range(NXB)]

        xm_f = [xm[i].rearrange("f g b t -> f (g b t)") for i in range(NB)]

        # prologue: critical pair (w g0-1, x g0 b0+b1) rides each ring solo;
        # g0's b2+b3 overlaps one element so its descriptors enter the ring
        # only after the first half completes (rings fair-share, not FIFO)
        nc.scalar.dma_start(xbufs[0][:, :XU], xm_f[0][:, :XU])
        nc.sync.dma_start(w_sb[:, :2 * kwc], wt[:, :2 * kwc])          # w g0-1
        nc.sync.dma_start(b_sb[:], bt[:])
        nc.scalar.dma_start(xbufs[0][:, XU - 1:XG], xm_f[0][:, XU - 1:XG])
        nc.sync.dma_start(w_sb[:, 2 * kwc:4 * kwc],
                          wt[:, 2 * kwc:4 * kwc])                      # w g2-3
        nc.sync.dma_start(xbufs[0][:, XG:], xm_f[0][:, XG:])           # g1
        nc.gpsimd.dma_start(xe_sb[:], xe[:])
        nc.gpsimd.dma_start(w_sb[:, 4 * kwc:], wt[:, 4 * kwc:])        # w g4-16

        for ib in range(NB):
            x_sb = xbufs[ib % NXB]
            om_f = om[ib].rearrange("o g b t -> o (g b t)")
            for j in range(GB):
                g = ib * GB + j
                o_sb = opool.tile([O, B * T], f16, tag="o")
                for b in range(B):
                    ps = pspool.tile([O, T], f32, tag="ps")
                    for k in range(K):
                        nc.tensor.matmul(
                            ps[:],
                            w_sb[:, (g * K + k) * O:(g * K + k + 1) * O],
                            x_sb[:, (j * B + b) * TP + k:
                                 (j * B + b) * TP + k + T],
                            start=(k == 0),
                            stop=(k == K - 1),
                        )
                    dst = o_sb[:, b * T:(b + 1) * T]
                    if b % 2 == 0:
                        nc.scalar.add(dst, ps[:], b_sb[:, g:g + 1])
                    else:
                        nc.vector.tensor_scalar_add(dst, ps[:],
                                                    b_sb[:, g:g + 1])
                    if b == 0:
                        # batch loads issue from inside the stream so their
                        # ring occupancy trails the critical transfers
                        if ib == 0 and j == 0:
                            nc.scalar.dma_start(xbufs[1][:], xm_f[1][:])
                        elif ib == 0 and j == 1:
                            nc.sync.dma_start(xbufs[2][:], xm_f[2][:])
                        elif j == 0 and 3 <= ib + 2 < NB:
                            e = nc.scalar if ib % 2 == 0 else nc.sync
                            e.dma_start(xbufs[(ib + 2) % NXB][:],
                                        xm_f[ib + 2][:])
                if g < GPC - 1:
                    e = nc.sync if g % 2 == 0 else nc.scalar
                    e.dma_start(om_f[:, j * B * T:(j + 1) * B * T], o_sb[:])
                else:
                    # last full group: store in pieces as the casts land so
                    # only b3's small store trails the matmul stream
                    c0 = j * B * T
                    nc.sync.dma_start(om_f[:, c0:c0 + 2 * T], o_sb[:, :2 * T])
                    nc.scalar.dma_start(om_f[:, c0 + 2 * T:c0 + 3 * T],
                                        o_sb[:, 2 * T:3 * T])
                    nc.sync.dma_start(om_f[:, c0 + 3 * T:c0 + 4 * T],
                                      o_sb[:, 3 * T:])

        # tail group (g=128) last: its drain is one [96,256] cast + 49 KB
        ps = pspool.tile([O, TE], f32, tag="ps")
        for k in range(K):
            nc.tensor.matmul(
                ps[:],
                w_sb[:, (GPC * K + k) * O:(GPC * K + k + 1) * O],
                xe_sb[:, k:k + TE],
                start=(k == 0),
                stop=(k == K - 1),
            )
        nc.scalar.add(oe_sb[:], ps[:], b_sb[:, GPC:GPC + 1])
        nc.scalar.dma_start(oe[:], oe_sb[:])

    nc.finalize()
    return nc


def shard_inputs(x, weight, bias):
    x = np.ascontiguousarray(x, dtype=np.float32)
    weight = np.ascontiguousarray(weight, dtype=np.float32)
    bias = np.ascontiguousarray(bias, dtype=np.float32)

    xp = np.pad(x, ((0, 0), (1, 1), (0, 0), (0, 0)))          # [B, TP, G, F]
    xt = xp.transpose(2, 3, 0, 1).astype(np.float16)          # [G, F, B, TP]
    # weight [G, O, F, K] -> [F, G, K, O]
    wtr = weight.transpose(2, 0, 3, 1).astype(np.float16)

    in_maps = []
    for c in range(NCORES):
        gs = list(range(c * GPC, (c + 1) * GPC)) + [G - 1]
        b_c, t0 = c // 2, (c % 2) * TE
        # [GPC, F, B, TP] -> [NB, GB, F, B, TP] -> [NB, F, GB, B, TP]
        xm_c = xt[c * GPC:(c + 1) * GPC].reshape(NB, GB, F, B, TP)
        in_maps.append({
            "xm": np.ascontiguousarray(xm_c.transpose(0, 2, 1, 3, 4)),
            "xe": np.ascontiguousarray(xt[G - 1, :, b_c, t0:t0 + TEP]),
            "wt": np.ascontiguousarray(wtr[:, gs].reshape(F, NG * K * O)),
            "bt": np.ascontiguousarray(bias[gs].T),
            })
    return in_maps


def unshard_outputs(results):
    out = np.empty((B, T, G, O), dtype=np.float32)
    for c in range(NCORES):
        om = results[c]["om"].astype(np.float32)        # [NB, O, GB, B, T]
        om = om.transpose(0, 2, 1, 3, 4).reshape(GPC, O, B, T)
        out[:, :, c * GPC:(c + 1) * GPC, :] = om.transpose(2, 3, 0, 1)
        b_c, t0 = c // 2, (c % 2) * TE
        out[b_c, t0:t0 + TE, G - 1, :] = results[c]["oe"].astype(np.float32).T
    return out


def run(x, weight, bias, **run_kwargs):
    nc = build_program()
    in_maps = shard_inputs(x, weight, bias)
    res = run_bass_kernel_spmd(nc, in_maps, list(range(NCORES)), **run_kwargs)
    return unshard_outputs(res.results), res


def kernel(x, weight, bias):
    out, _ = run(x, weight, bias)
    return out
